# revision 33
# baseline (speedup 1.0000x reference)
"""ArcMargin softmax-with-loss on 8 TRN2 NeuronCores (Bass/Tile).

Strategy (model-parallel softmax cross-entropy):
  - Shard num_classes (axis 1) across 8 cores: each core holds a
    [512, 12500] f32 slice of cos_theta.
  - Since S*cos_theta is bounded by S=30, use a FIXED logsumexp shift of
    30 (exp(S*x-30) in (e^-60, 1]) -> no rowwise-max all-reduce needed.
  - Each core streams its 25.6MB shard once: ScalarE activation
    exp(S*x - 30) with accum_out produces rowwise partial sums fused
    with the elementwise pass (DMA-bound, ~358 GB/s/core roofline).
  - The target gather is a one-hot masked reduce fused into one DVE
    scalar_tensor_tensor per chunk: (iota == local_col) * x, accum.
  - ArcMargin correction per 128-row block (sqrt via DVE Newton rsqrt,
    so no ACT table switches during streaming).
  - Two small AllReduces: blocks 0-1 launch mid-stream (absorb
    collective-engine init + inter-core launch skew under streaming);
    blocks 2-3 at the end. Every core computes
    loss = mean(log(Z') + 30 - S*phi_target) redundantly.

GAMMA=0 in the reference makes (1-pt)^GAMMA == 1, so
loss = mean_b(logsumexp_c(out[b,:]) - out[b, t_b]).
"""

import math

import numpy as np

import concourse.bacc as bacc
import concourse.bass as bass  # noqa: F401  (kept for AP helpers)
import concourse.tile as tile
from concourse.tile import add_dep_helper
from concourse import mybir
from concourse import bass_utils

S = 30.0
M = 0.5
COS_M = math.cos(M)
SIN_M = math.sin(M)
TH = math.cos(math.pi - M)
MM = math.sin(math.pi - M) * M
SHIFT = 30.0  # fixed logsumexp shift; S*cos_theta <= 30

N_CORES = 8
B = 512
C = 100000
C_LOC = C // N_CORES  # 12500
P = 128
NBLK = B // P  # 4 row blocks of 128
# Per-block column chunking. Block 0 starts tiny so compute starts as soon
# as possible; block 3 ends tiny so the compute trail past the last DMA is
# short.
BLOCK_SPLITS = [
    [625, 1875, 2500, 2500, 2500, 2500],
    [2500, 2500, 2500, 2500, 2500],
    [2500, 2500, 2500, 2500, 2500],
    [2500, 2500, 2500, 2500, 1875, 625],
]
F32 = mybir.dt.float32
F16 = mybir.dt.float16
I16 = mybir.dt.int16
I32 = mybir.dt.int32
U32 = mybir.dt.uint32
AF = mybir.ActivationFunctionType
ALU = mybir.AluOpType


def build(c_loc=C_LOC, block_splits=None, stream_bufs=6, single_ar=False,
          iota_const=True, f16=False):
    bs = block_splits or BLOCK_SPLITS
    assert len(bs) == NBLK and all(sum(s) == c_loc for s in bs)
    ncols = sum(len(s) for s in bs)
    col0 = [sum(len(s) for s in bs[:k]) for k in range(NBLK)]
    max_chunk = max(max(s) for s in bs)

    nc = bacc.Bacc(
        "TRN2", target_bir_lowering=False, debug=False, num_devices=N_CORES
    )
    xdt = F16 if f16 else F32
    idt = I16 if f16 else I32
    x = nc.dram_tensor("x", [B * c_loc], xdt, kind="ExternalInput")
    # lcoff[p, col0[k]+j] = target's column within chunk j of block k's row
    # (k*128+p) if this core owns that row's target and it falls in that
    # chunk, else -1 (matches no iota value -> one-hot mask is all-zero).
    lcoff = nc.dram_tensor("lcoff", [P, ncols], idt, kind="ExternalInput")
    mask = nc.dram_tensor("mask", [P, NBLK], F32, kind="ExternalInput")
    out = nc.dram_tensor("out", [1, 1], F32, kind="ExternalOutput")

    x3 = x.ap().rearrange("(k p c) -> k p c", p=P, c=c_loc)  # [NBLK, P, c_loc]

    with tile.TileContext(nc) as tc:
        with (
            tc.tile_pool(name="stream", bufs=stream_bufs) as stream,
            tc.tile_pool(name="mscratch", bufs=2) as mscratch,
            tc.tile_pool(name="escratch", bufs=2) as escratch,
            tc.tile_pool(name="small", bufs=1) as small,
            tc.tile_pool(name="dram", bufs=1, space="DRAM") as dram,
            tc.tile_pool(name="psum", bufs=1, space="PSUM") as psum,
        ):
            lcoff_sb = small.tile([P, ncols], idt)
            mask_sb = small.tile([P, NBLK], F32)
            # small metadata inputs ride the gpsimd queue so the in-order
            # sync queue can issue stream chunk 0 immediately.
            nc.gpsimd.dma_start(out=lcoff_sb[:], in_=lcoff.ap())
            nc.gpsimd.dma_start(out=mask_sb[:], in_=mask.ap())

            nbias = small.tile([P, 1], F32)  # bias AP = -SHIFT for Exp calls
            nc.vector.memset(nbias[:], -SHIFT)

            # iota as a baked-in Const row, broadcast-DMA'd across the 128
            # partitions: a gpsimd InstIota takes 4-5us and gates the first
            # masked reduce; this lands in ~2us.
            iota_sb = small.tile([P, max_chunk], idt)
            if iota_const:
                iota_row = nc.inline_tensor(
                    np.arange(max_chunk, dtype=mybir.dt.np(idt)).reshape(1, -1),
                    name="iota_row",
                )
                nc.sync.dma_start(
                    out=iota_sb[:],
                    in_=iota_row.ap().to_broadcast([P, max_chunk]),
                )
            else:
                nc.gpsimd.iota(
                    iota_sb[:], pattern=[[1, max_chunk]], base=0,
                    channel_multiplier=0,
                )

            acc = small.tile([P, ncols], F32)
            xacc = small.tile([P, ncols], F32)
            zp = small.tile([P, NBLK], F32)
            xg = small.tile([P, NBLK], F32)
            s2 = small.tile([P, NBLK], F32)
            sh = small.tile([P, NBLK], U32)
            r_u = small.tile([P, NBLK], U32)
            t1 = small.tile([P, NBLK], F32)
            phi = small.tile([P, NBLK], F32)
            alt = small.tile([P, NBLK], F32)
            cond = small.tile([P, NBLK], I32)  # CopyPredicated needs int mask
            e1 = small.tile([P, NBLK], F32)
            e2 = small.tile([P, NBLK], F32)
            ar_a = small.tile([P, 4], F32)  # [Z'0, Z'1, tgt0, tgt1]
            ar_b = small.tile([P, 4], F32)  # [Z'2, Z'3, tgt2, tgt3]
            epi_exps = []  # ACT instrs feeding the ARs; must precede the lns

            def block_epilogue(k):
                # per-block ArcMargin correction on [P,1] columns; emitted
                # right after block k's chunks so it hides in engine slack
                # under the remaining streaming.
                c = slice(k, k + 1)
                nch_k = len(bs[k])
                nc.vector.tensor_reduce(
                    zp[:, c], acc[:, col0[k] : col0[k] + nch_k],
                    axis=mybir.AxisListType.X, op=ALU.add,
                )
                nc.vector.tensor_reduce(
                    xg[:, c], xacc[:, col0[k] : col0[k] + nch_k],
                    axis=mybir.AxisListType.X, op=ALU.add,
                )
                # sin = sqrt(relu(1 - x^2)) via DVE-only Newton rsqrt
                # (avoids sqrt/ln ACT table loads thrashing the exp set).
                nc.vector.tensor_mul(s2[:, c], xg[:, c], xg[:, c])
                nc.vector.tensor_scalar(
                    s2[:, c], s2[:, c], -1.0, 1.0, ALU.mult, ALU.add
                )
                nc.vector.tensor_scalar_max(s2[:, c], s2[:, c], 0.0)
                # rsqrt seed: r = bitcast(0x5f3759df - (bits(s2) >> 1))
                nc.vector.tensor_scalar(
                    sh[:, c], s2[:, c].bitcast(U32), 1, None,
                    ALU.logical_shift_right,
                )
                nc.vector.tensor_scalar(
                    r_u[:, c], sh[:, c], -1.0, float(0x5F3759DF),
                    ALU.mult, ALU.add,
                )
                r = r_u[:, c].bitcast(F32)
                # 2 Newton iterations: seed err 3.5e-2 -> 1.8e-3 -> 4.9e-6
                # relative on rsqrt; ~1e-7 effect on the final loss.
                for _ in range(2):  # r *= 1.5 - 0.5*s2*r*r
                    nc.vector.tensor_mul(t1[:, c], r, r)
                    nc.vector.tensor_mul(t1[:, c], t1[:, c], s2[:, c])
                    nc.vector.tensor_scalar(
                        t1[:, c], t1[:, c], -0.5, 1.5, ALU.mult, ALU.add
                    )
                    nc.vector.tensor_mul(r, r, t1[:, c])
                nc.vector.tensor_mul(t1[:, c], r, s2[:, c])  # sin
                nc.vector.tensor_scalar(t1[:, c], t1[:, c], SIN_M, None, ALU.mult)
                nc.vector.tensor_scalar(phi[:, c], xg[:, c], COS_M, None, ALU.mult)
                nc.vector.tensor_sub(phi[:, c], phi[:, c], t1[:, c])
                nc.vector.tensor_scalar(alt[:, c], xg[:, c], -MM, None, ALU.add)
                nc.vector.tensor_scalar(cond[:, c], xg[:, c], TH, None, ALU.is_le)
                nc.vector.copy_predicated(phi[:, c], cond[:, c], alt[:, c])
                epi_exps.append(nc.scalar.activation(
                    e1[:, c], phi[:, c], AF.Exp, bias=nbias[:], scale=S
                ))
                epi_exps.append(nc.scalar.activation(
                    e2[:, c], xg[:, c], AF.Exp, bias=nbias[:], scale=S
                ))
                nc.vector.tensor_sub(e1[:, c], e1[:, c], e2[:, c])
                nc.vector.tensor_mul(e1[:, c], e1[:, c], mask_sb[:, c])  # corr
                nc.vector.tensor_scalar(t1[:, c], phi[:, c], S, None, ALU.mult)
                nc.vector.tensor_mul(t1[:, c], t1[:, c], mask_sb[:, c])  # tgt
                half, cpos = (ar_a, k) if k < 2 else (ar_b, k - 2)
                nc.vector.tensor_add(
                    half[:, cpos : cpos + 1], zp[:, c], e1[:, c]
                )
                nc.vector.tensor_copy(
                    half[:, 2 + cpos : 3 + cpos], t1[:, c]
                )

            cc_in_a = dram.tile([P, 4], F32)
            cc_out_a = dram.tile([P, 4], F32)
            cc_in_b = dram.tile([P, 4], F32)
            cc_out_b = dram.tile([P, 4], F32)

            if single_ar:
                # v3-style: const-input warmup AR at t=0 absorbs collective
                # init + skew; one combined AR at the end.
                warm_in = nc.inline_tensor(
                    np.zeros((P, 1), dtype=np.float32), name="warm_zeros"
                )
                warm_out = dram.tile([P, 1], F32)
                nc.gpsimd.collective_compute(
                    "AllReduce",
                    ALU.add,
                    replica_groups=[list(range(N_CORES))],
                    ins=[warm_in.ap().opt()],
                    outs=[warm_out.opt()],
                )

            # --- streaming pass ------------------------------------------
            # Per chunk: ACT computes exp(S*x-30) -> scratch with rowwise
            # accum (partial Z); DVE extracts the target element via a
            # fused one-hot masked reduce. Both only READ the streamed
            # tile, so they run concurrently.
            for k in range(NBLK):
                off = 0
                for j, w in enumerate(bs[k]):
                    t = stream.tile([P, w], xdt, tag="stream")
                    nc.sync.dma_start(out=t[:], in_=x3[k, :, off : off + w])
                    off += w
                    col = col0[k] + j
                    m = mscratch.tile([P, w], xdt, tag="m")
                    nc.vector.scalar_tensor_tensor(
                        out=m[:],
                        in0=iota_sb[:, :w],
                        scalar=lcoff_sb[:, col : col + 1],
                        in1=t[:],
                        op0=ALU.is_equal,
                        op1=ALU.mult,
                        accum_out=xacc[:, col : col + 1],
                    )
                    e_t = escratch.tile([P, w], xdt, tag="e")
                    nc.scalar.activation(
                        e_t[:],
                        t[:],
                        AF.Exp,
                        bias=nbias[:],
                        scale=S,
                        accum_out=acc[:, col : col + 1],
                    )
                block_epilogue(k)
                if k == 1 and not single_ar:
                    # First AllReduce launches mid-stream: its entry barrier
                    # absorbs collective-engine init and inter-core launch
                    # skew while blocks 2-3 are still streaming. Its input
                    # DMA rides the idle gpsimd queue so the in-order sync
                    # queue keeps issuing stream chunks without a hiccup.
                    nc.gpsimd.dma_start(out=cc_in_a[:], in_=ar_a[:])
                    nc.gpsimd.collective_compute(
                        "AllReduce",
                        ALU.add,
                        replica_groups=[list(range(N_CORES))],
                        ins=[cc_in_a.opt()],
                        outs=[cc_out_a.opt()],
                    )

            g = small.tile([P, 8], F32)
            if single_ar:
                cc_in = dram.tile([P, 8], F32)
                cc_out = dram.tile([P, 8], F32)
                nc.gpsimd.dma_start(out=cc_in[:, 0:4], in_=ar_a[:])
                nc.gpsimd.dma_start(out=cc_in[:, 4:8], in_=ar_b[:])
                nc.gpsimd.collective_compute(
                    "AllReduce",
                    ALU.add,
                    replica_groups=[list(range(N_CORES))],
                    ins=[cc_in.opt()],
                    outs=[cc_out.opt()],
                )
                nc.gpsimd.dma_start(out=g[:], in_=cc_out[:])
            else:
                nc.gpsimd.dma_start(out=cc_in_b[:], in_=ar_b[:])
                nc.gpsimd.collective_compute(
                    "AllReduce",
                    ALU.add,
                    replica_groups=[list(range(N_CORES))],
                    ins=[cc_in_b.opt()],
                    outs=[cc_out_b.opt()],
                )
                # readbacks on the gpsimd queue (idle here); sync queue
                # stays free for the final output DMA only.
                nc.gpsimd.dma_start(out=g[:, 0:4], in_=cc_out_a[:])
                nc.gpsimd.dma_start(out=g[:, 4:8], in_=cc_out_b[:])

            # --- loss = mean(log(Z') + SHIFT - tgt) -----------------------
            lg = small.tile([P, 4], F32)
            t2 = small.tile([P, 4], F32)
            # halves: g = [Z'0 Z'1 tgt0 tgt1 | Z'2 Z'3 tgt2 tgt3]
            ln1 = nc.scalar.activation(lg[:, 0:2], g[:, 0:2], AF.Ln)
            # ACT is in-order: without explicit edges the scheduler may
            # place this Ln (which waits on AllReduce A's result) BEFORE
            # the block-3 exps that feed AllReduce B, deadlocking B's
            # launch behind A's completion (~10us). Pin the order.
            for bi in epi_exps:
                add_dep_helper(ln1.ins, bi.ins, sync=False,
                               reason="AR inputs before post-AR ln")
            nc.vector.tensor_sub(t2[:, 0:2], lg[:, 0:2], g[:, 2:4])
            ln2 = nc.scalar.activation(lg[:, 2:4], g[:, 4:6], AF.Ln)
            add_dep_helper(ln2.ins, ln1.ins, sync=False,
                           reason="ln order follows AR order")
            nc.vector.tensor_sub(t2[:, 2:4], lg[:, 2:4], g[:, 6:8])
            r1 = small.tile([P, 1], F32)
            nc.vector.tensor_reduce(
                r1[:], t2[:], axis=mybir.AxisListType.X, op=ALU.add
            )
            ones = small.tile([P, 1], F32)
            nc.vector.memset(ones[:], 1.0)
            ps = psum.tile([1, 1], F32)
            nc.tensor.matmul(ps[:], lhsT=r1[:], rhs=ones[:], start=True, stop=True)
            loss = small.tile([1, 1], F32)
            nc.vector.tensor_scalar(
                loss[:], ps[:], 1.0 / B, SHIFT, ALU.mult, ALU.add
            )
            nc.sync.dma_start(out=out.ap(), in_=loss[:])
    nc.finalize()
    return nc


def prep_in_maps(cos_theta, target, c_loc=C_LOC, block_splits=None,
                 n_cores=N_CORES, f16=False):
    bs = block_splits or BLOCK_SPLITS
    assert len(bs) == NBLK and all(sum(s) == c_loc for s in bs)
    ncols = sum(len(s) for s in bs)
    col0 = [sum(len(s) for s in bs[:k]) for k in range(NBLK)]
    cos_theta = np.ascontiguousarray(np.asarray(cos_theta), dtype=np.float32)
    target = np.asarray(target).astype(np.int64)
    in_maps = []
    for i in range(n_cores):
        lo = i * c_loc
        sh = np.ascontiguousarray(cos_theta[:, lo : lo + c_loc]).reshape(-1)
        if f16:
            sh = sh.astype(np.float16)
        local = (target >= lo) & (target < lo + c_loc)
        li = np.where(local, target - lo, -1)  # [B] col within shard or -1
        li_pk = li.reshape(NBLK, P).T  # [P, NBLK]; row k*128+p -> [p, k]
        lc = np.full((P, ncols), -1, dtype=np.int64)
        for k in range(NBLK):
            off = 0
            for j, w in enumerate(bs[k]):
                o = li_pk[:, k] - off
                hit = (o >= 0) & (o < w) & (li_pk[:, k] >= 0)
                lc[hit, col0[k] + j] = o[hit]
                off += w
        msk = np.ascontiguousarray(local.reshape(NBLK, P).T).astype(np.float32)
        in_maps.append(
            {
                "x": sh,
                "lcoff": np.ascontiguousarray(lc).astype(
                    np.int16 if f16 else np.int32
                ),
                "mask": msk,
            }
        )
    return in_maps


_CACHE = {}


def _get_nc(single_ar=False, f16=False):
    key = ("nc", single_ar, f16)
    if key not in _CACHE:
        _CACHE[key] = build(single_ar=single_ar, f16=f16)
    return _CACHE[key]


def run(cos_theta, target, trace=False, single_ar=False, f16=False):
    """Returns (loss ndarray shape (), exec_time_ns or None)."""
    nc = _get_nc(single_ar, f16)
    in_maps = prep_in_maps(cos_theta, target, f16=f16)
    res = bass_utils.run_bass_kernel_spmd(
        nc, in_maps, core_ids=list(range(N_CORES)), trace=trace
    )
    loss = np.asarray(res.results[0]["out"], dtype=np.float32).reshape(())
    return loss, res.exec_time_ns


def kernel(cos_theta, target):
    loss, _ = run(cos_theta, target)
    return loss
